# revision 39
# baseline (speedup 1.0000x reference)
"""Trainium2 Bass kernel for nn_Attention_84026740179215.

Multi-head attention: x[8,1024,768] -> qkv -> per-head softmax(QK^T/sqrt(d))V -> proj.
Sharding: pure data parallel, one batch element per NeuronCore (B=8 = 8 cores).

v3 layout (N=1024 tokens, C=768, H=12 heads, D=64):
  - Heads processed in PAIRS (2p, 2p+1). Q^T/K^T chunks [128, N] hold the even
    head in partitions 0:64 and the odd head in 64:128, so the two QK^T
    matmuls (K=64 each) run CONCURRENTLY on the PE via row tiling
    (tile_position (0,0) / (64,0)).
  - All matmul lhsT (stationary) operands are float32r -> self-loading
    matmuls, no separate LDWEIGHTS instruction on the PE sequencer.
    Moving operands that are never lhsT (q chunks, P=exp(S), w_v, w_proj)
    are bf16 to save SBUF/DMA.
  - exp via ACT (scale=1/8 fused) PSUM->SBUF per head tile [128, 1024].
  - V augmented with a ones column per head: PV (lhsT=vaug[m,65], rhs=P^T)
    also produces the softmax denominator Z in row 64.
  - Normalization by 1/Z via reciprocal_approx_fast + gpsimd
    partition_broadcast, written into the projection's lhsT layout [c', n].
  - QKV chunk production threads through the attention pairs as half-blocks
    ([128,512] PSUM accumulators borrowing the st slot rotation); V production
    runs just-in-time inside pair 0. The first two chunks (pair 0's Q,K) run
    kt-major off small per-kt weight DMAs so PE work starts ~2us in.
"""

import numpy as np
import ml_dtypes

import concourse.bacc as bacc
import concourse.bass as bass
import concourse.mybir as mybir
import concourse.tile as tile
from concourse import bass_utils

N_CORES = 8
N = 1024          # tokens per batch element
C = 768           # model dim
H = 12            # heads
D = 64            # head dim
KT = C // 128     # 6 k-tiles of the contraction dim
NCH = N // 128    # 8 chunks of the token dim (query side)
MT = N // 128     # 8 tiles of the token dim (key/value side)
NPAIR = H // 2    # 6 head pairs

DEBUG_TAPS = False
BENCH_ITERS = 0      # >0: wrap the body in a For_i loop (timing harness only)
F32 = mybir.dt.float32
F32R = mybir.dt.float32r
BF16 = mybir.dt.bfloat16
AF = mybir.ActivationFunctionType
NP_BF16 = ml_dtypes.bfloat16


def _build():
    nc = bacc.Bacc("TRN2", target_bir_lowering=False, debug=False,
                   num_devices=N_CORES)

    xT = nc.dram_tensor("xT", [C, N], F32R, kind="ExternalInput")
    w_qk = nc.dram_tensor("w_qk", [C, 2 * C], F32R, kind="ExternalInput")
    w_v = nc.dram_tensor("w_v", [C, C], F32R, kind="ExternalInput")
    w_proj = nc.dram_tensor("w_proj", [C, C], F32R, kind="ExternalInput")
    b_qk = nc.dram_tensor("b_qk", [2 * KT, 128, 1], F32, kind="ExternalInput")
    b_vo = nc.dram_tensor("b_vo", [2, C], F32, kind="ExternalInput")
    ones12 = nc.dram_tensor("ones12", [128, H], F32R, kind="ExternalInput")
    y = nc.dram_tensor("y", [N, C], F32, kind="ExternalOutput")
    dbg = {}

    with tile.TileContext(nc) as tc:
        if BENCH_ITERS > 0:
            with tc.For_i(0, BENCH_ITERS, 1):
                _body(nc, tc, xT, w_qk, w_v, w_proj, b_qk, b_vo, ones12, y, dbg)
        else:
            _body(nc, tc, xT, w_qk, w_v, w_proj, b_qk, b_vo, ones12, y, dbg)
    nc.compile()
    return nc


def _body(nc, tc, xT, w_qk, w_v, w_proj, b_qk, b_vo, ones12, y, dbg={}):
    import contextlib
    ctx = contextlib.ExitStack()
    with ctx:
        # ---- pools ----
        xt_pool = ctx.enter_context(tc.tile_pool(name="xt", bufs=1))
        wq_pool = ctx.enter_context(tc.tile_pool(name="wq", bufs=1))
        wv_pool = ctx.enter_context(tc.tile_pool(name="wv", bufs=1))
        qc_pool = ctx.enter_context(tc.tile_pool(name="qc", bufs=1))
        kc_pool = ctx.enter_context(tc.tile_pool(name="kc", bufs=1))
        pts_pool = ctx.enter_context(tc.tile_pool(name="pts", bufs=6))
        vaug_pool = ctx.enter_context(tc.tile_pool(name="vaug", bufs=1))
        onorm_pool = ctx.enter_context(tc.tile_pool(name="onorm", bufs=1))
        bias_pool = ctx.enter_context(tc.tile_pool(name="bias", bufs=1))
        u_pool = ctx.enter_context(tc.tile_pool(name="u", bufs=1))
        zrow_pool = ctx.enter_context(tc.tile_pool(name="zrow", bufs=1))
        rz_pool = ctx.enter_context(tc.tile_pool(name="rz", bufs=1))
        rzb_pool = ctx.enter_context(tc.tile_pool(name="rzb", bufs=1))
        ysb_pool = ctx.enter_context(tc.tile_pool(name="ysb", bufs=2))
        # PSUM: tag "st" 2x[128,1024]f32 + tag "oa" 2x[128,1024]f32 = all 8 banks
        ps = ctx.enter_context(tc.tile_pool(name="ps", bufs=2, space="PSUM"))

        # ---- input staging ----
        # Mini weight tiles for pair 0's chunks (cc=0 -> w_qk cols 0:128,
        # cc=KT -> cols 768:896): two plain [128, 128] DMAs per kt, first on
        # the queues so the kt-major bootstrap can start early.
        wmini = []
        xq = [nc.sync, nc.scalar, nc.gpsimd]
        for kt in range(KT):
            t = wq_pool.tile([128, 256], F32R, tag=f"wq{kt}", name=f"wm{kt}",
                             padded_shape=[128, 2 * C])
            xq[kt % 3].dma_start(t[:, 0:128],
                                 w_qk.ap()[kt * 128:(kt + 1) * 128, 0:128])
            xq[kt % 3].dma_start(t[:, 128:256],
                                 w_qk.ap()[kt * 128:(kt + 1) * 128, C:C + 128])
            wmini.append(t)
        # xT k-tiles, split in halves to pipeline the kt-major bootstrap.
        xt_sb = []
        for kt in range(KT):
            t = xt_pool.tile([128, N], F32R, tag=f"xt{kt}", name=f"xt{kt}")
            xq[kt % 3].dma_start(t[:, 0:512],
                                 xT.ap()[kt * 128:(kt + 1) * 128, 0:512])
            xq[(kt + 1) % 3].dma_start(t[:, 512:1024],
                                       xT.ap()[kt * 128:(kt + 1) * 128, 512:1024])
            xt_sb.append(t)
        # biases (b_qk per-partition scalars; b_v/b_o broadcast on-chip)
        bqk_sb = {}
        for cc in range(2 * KT):
            t = bias_pool.tile([128, 1], F32, tag=f"bqk{cc}", name=f"bqk{cc}")
            nc.gpsimd.dma_start(t[:], b_qk.ap()[cc])
            bqk_sb[cc] = t
        bv_row = bias_pool.tile([1, C], F32, tag="bvrow", name="bv_row")
        nc.gpsimd.dma_start(bv_row[:], b_vo.ap()[0:1, :])
        bo_row = bias_pool.tile([1, C], F32, tag="borow", name="bo_row")
        nc.gpsimd.dma_start(bo_row[:], b_vo.ap()[1:2, :])
        bv_sb = bias_pool.tile([128, C], F32, tag="bv", name="bv")
        nc.gpsimd.partition_broadcast(bv_sb[:], bv_row[:])
        bo_sb = bias_pool.tile([128, C], F32, tag="bo", name="bo")
        nc.gpsimd.partition_broadcast(bo_sb[:], bo_row[:])
        ones_sb = bias_pool.tile([128, H], F32R, tag="ones", name="ones")
        nc.gpsimd.dma_start(ones_sb[:], ones12.ap())
        # w_v k-tiles first (pair-0 V pass), then full w_qk tiles (their DMAs
        # block on the wmini slot until the bootstrap's kt stage reads finish).
        wv_sb = []
        for kt in range(KT):
            t2 = wv_pool.tile([128, C], F32R, tag=f"wv{kt}", name=f"wv{kt}")
            xq[(kt + 2) % 3].dma_start(
                t2[:], w_v.ap()[kt * 128:(kt + 1) * 128, :])
            wv_sb.append(t2)
        wq_sb = []
        for kt in range(KT):
            t = wq_pool.tile([128, 2 * C], F32R, tag=f"wq{kt}", name=f"wq{kt}")
            xq[(kt + 1) % 3].dma_start(
                t[:], w_qk.ap()[kt * 128:(kt + 1) * 128, :])
            wq_sb.append(t)

        # q and k chunks bf16: the row-tiled st matmuls need the LDWEIGHTS
        # path (tile_position), which silently breaks for f32r weights on HW.
        qc_sb = [qc_pool.tile([128, N], BF16, tag=f"qc{p}", name=f"qc{p}")
                 for p in range(NPAIR)]
        kc_sb = [kc_pool.tile([128, N], BF16, tag=f"kc{p}", name=f"kc{p}")
                 for p in range(NPAIR)]

        def chunk_half(cc, half, wsrc=None):
            """Produce chunk cc's columns [half*512:(half+1)*512]."""
            s = slice(half * 512, (half + 1) * 512)
            pc = ps.tile([128, 512], F32, tag="st", name=f"pc{cc}_{half}")
            for kt in range(KT):
                w = wsrc[kt] if wsrc else wq_sb[kt][:, cc * 128:(cc + 1) * 128]
                nc.tensor.matmul(pc[:], w, xt_sb[kt][:, s],
                                 start=(kt == 0), stop=(kt == KT - 1))
            dst = qc_sb[cc] if cc < KT else kc_sb[cc - KT]
            nc.vector.tensor_scalar_add(dst[:, s], pc[:], bqk_sb[cc][:])

        vaug_sb = [None] * MT

        def make_vaug(mt):
            """V rows for m-tile mt, augmented with a ones column per head."""
            vc = ps.tile([128, C], F32, tag="st", name=f"vc{mt}")
            for kt in range(KT):
                nc.tensor.matmul(vc[:, 0:512],
                                 xt_sb[kt][:, mt * 128:(mt + 1) * 128],
                                 wv_sb[kt][:, 0:512],
                                 start=(kt == 0), stop=(kt == KT - 1))
                nc.tensor.matmul(vc[:, 512:768],
                                 xt_sb[kt][:, mt * 128:(mt + 1) * 128],
                                 wv_sb[kt][:, 512:768],
                                 start=(kt == 0), stop=(kt == KT - 1))
            va = vaug_pool.tile([128, H * (D + 1)], F32R, tag=f"vaug{mt}",
                                name=f"vaug{mt}")
            va_h = va[:].rearrange("p (h s) -> p h s", h=H)
            nc.vector.tensor_copy(va_h[:, :, D], ones_sb[:])
            nc.vector.tensor_add(
                va_h[:, :, 0:D],
                vc[:].rearrange("p (h s) -> p h s", h=H),
                bv_sb[:].rearrange("p (h s) -> p h s", h=H))
            vaug_sb[mt] = va

        onorm_sb = [onorm_pool.tile([128, N], F32R, tag=f"onorm{p}",
                                    name=f"onorm{p}")
                    for p in range(NPAIR)]

        class Pair:
            """Heads h0=2p (partitions 0:64) and h1=2p+1 (64:128)."""

            def __init__(self, p):
                self.p = p
                self.qt = qc_sb[p]
                self.kt = kc_sb[p]
                self.oa = [ps.tile([128, N], F32, tag="oa", name=f"oa{2*p+i}")
                           for i in range(2)]
                self.pts = [[None] * MT, [None] * MT]

            def st_exp(self, mc):
                st = [ps.tile([128, N], F32, tag="st",
                              name=f"st{2*self.p+i}_{mc}") for i in range(2)]
                ms = slice(mc * 128, (mc + 1) * 128)
                for half in range(2):
                    s = slice(half * 512, (half + 1) * 512)
                    nc.tensor.matmul(st[0][:, s], self.kt[0:64, ms],
                                     self.qt[0:64, s], start=True, stop=True,
                                     tile_position=(0, 0))
                    nc.tensor.matmul(st[1][:, s], self.kt[64:128, ms],
                                     self.qt[64:128, s], start=True, stop=True,
                                     tile_position=(64, 0))
                for i in range(2):
                    pt = pts_pool.tile([128, N], F32R, tag="pts",
                                       name=f"pt{2*self.p+i}_{mc}")
                    nc.scalar.activation(pt[:], st[i][:], AF.Exp,
                                         scale=float(D) ** -0.5)
                    self.pts[i][mc] = pt

            def pv(self, mc):
                for i in range(2):
                    h = 2 * self.p + i
                    for half in range(2):
                        s = slice(half * 512, (half + 1) * 512)
                        nc.tensor.matmul(
                            self.oa[i][0:D + 1, s],
                            vaug_sb[mc][:, h * (D + 1):(h + 1) * (D + 1)],
                            self.pts[i][mc][:, s],
                            start=(mc == 0), stop=(mc == MT - 1))

            def norm(self, i):
                h, oa = 2 * self.p + i, self.oa[i]
                u = u_pool.tile([D, N], BF16, tag="u", name=f"u{h}")
                nc.vector.tensor_copy(u[:], oa[0:D, :])
                zrow = zrow_pool.tile([1, N], F32, tag="zrow", name=f"z{h}")
                nc.vector.tensor_copy(zrow[:], oa[D:D + 1, :])
                rz = rz_pool.tile([1, N], F32, tag="rz", name=f"rz{h}")
                nc.vector.reciprocal_approx_fast(rz[:], zrow[:])
                rzb = rzb_pool.tile([D, N], F32, tag="rzb", name=f"rzb{h}")
                nc.gpsimd.partition_broadcast(rzb[:], rz[:])
                nc.vector.tensor_mul(
                    onorm_sb[self.p][i * D:(i + 1) * D, :], u[:], rzb[:])

        def do_pair(p, fillers=(), lookahead=2):
            pr = Pair(p)
            fi = iter(fillers)
            for mc in range(MT):
                pr.st_exp(mc)
                f = next(fi, None)
                if f:
                    f()
                if mc >= lookahead:
                    pr.pv(mc - lookahead)
            for f in fi:
                f()
            for mc in range(MT - lookahead, MT):
                pr.pv(mc)
            pr.norm(0)
            pr.norm(1)
            return pr

        # ---- bootstrap: pair 0's chunks kt-major off the mini weight DMAs ----
        # three accumulators in flight: (0,h0)+(0,h1) on "st", (KT,h0) on "oa"
        bpc = {}
        bpc[(0, 0)] = ps.tile([128, 512], F32, tag="st", name="bpc00")
        bpc[(0, 1)] = ps.tile([128, 512], F32, tag="st", name="bpc01")
        bpc[(KT, 0)] = ps.tile([128, 512], F32, tag="oa", name="bpc60")
        for kt in range(KT):
            for (cc, half), pc in bpc.items():
                s = slice(half * 512, (half + 1) * 512)
                w = wmini[kt][:, 0:128] if cc == 0 else wmini[kt][:, 128:256]
                nc.tensor.matmul(pc[:], w, xt_sb[kt][:, s],
                                 start=(kt == 0), stop=(kt == KT - 1))
        for (cc, half), pc in bpc.items():
            dst = qc_sb[cc] if cc < KT else kc_sb[cc - KT]
            s = slice(half * 512, (half + 1) * 512)
            nc.vector.tensor_scalar_add(dst[:, s], pc[:], bqk_sb[cc][:])

        # pair 0: V production JIT + remaining chunk halves as PE fillers
        f0 = [lambda: chunk_half(KT, 1)] + \
             [lambda mt=mt: make_vaug(mt) for mt in range(MT)] + \
             [lambda: chunk_half(1, 0), lambda: chunk_half(1, 1),
              lambda: chunk_half(KT + 1, 0)]
        do_pair(0, f0)

        # w_proj staging: reuses the wq slots (frees after the last chunk
        # reads wq{kt}, i.e. during pair 4; proj needs them in pair 5+).
        wp_sb = []
        for kt in range(KT):
            t = wq_pool.tile([128, C], F32R, tag=f"wq{kt}", name=f"wp{kt}",
                             padded_shape=[128, 2 * C])
            nc.sync.dma_start(t[:], w_proj.ap()[kt * 128:(kt + 1) * 128, :])
            wp_sb.append(t)

        # partial projection over pairs 0..4 (runs as pair-5 fillers in the
        # PE slack under the ACT-bound cadence); spilled to SBUF.
        ypart_sb = [None] * NCH

        def proj_partial(nch):
            yp = ps.tile([128, C], F32, tag="st", name=f"ypp{nch}")
            for kt in range(KT - 1):
                nc.tensor.matmul(yp[:, 0:512],
                                 onorm_sb[kt][:, nch * 128:(nch + 1) * 128],
                                 wp_sb[kt][:, 0:512],
                                 start=(kt == 0), stop=(kt == KT - 2))
                nc.tensor.matmul(yp[:, 512:768],
                                 onorm_sb[kt][:, nch * 128:(nch + 1) * 128],
                                 wp_sb[kt][:, 512:768],
                                 start=(kt == 0), stop=(kt == KT - 2))
            # spill into a dead q/k-chunk slot (chunk tiles are read for the
            # last time during their own pair's st matmuls)
            if nch < NPAIR:
                t = qc_pool.tile([128, C], BF16, tag=f"qc{nch}",
                                 name=f"ypart{nch}", padded_shape=[128, N])
            else:
                t = kc_pool.tile([128, C], BF16, tag=f"kc{nch - NPAIR}",
                                 name=f"ypart{nch}", padded_shape=[128, N])
            nc.vector.tensor_add(t[:], yp[:], bo_sb[:])
            ypart_sb[nch] = t

        # steady pairs: pair p fillers produce K(p)h1 + Q(p+1) + K(p+1)h0;
        # pair 5 fillers run the partial projection instead.
        for p in range(1, NPAIR):
            fillers = [lambda p=p: chunk_half(KT + p, 1)]
            if p + 1 < NPAIR:
                fillers += [lambda p=p: chunk_half(p + 1, 0),
                            lambda p=p: chunk_half(p + 1, 1),
                            lambda p=p: chunk_half(KT + p + 1, 0)]
            else:
                # ypp5 must trail pair 5's st(7) (it reuses qc5's slot)
                fillers += [lambda n=n: proj_partial(n)
                            for n in (0, 1, 2, 3, 4, 6, 5)]
            do_pair(p, fillers, lookahead=2 if p < NPAIR - 1 else 3)
        proj_partial(NCH - 1)

        # ---- final projection pass: += onorm[KT-1]^T @ wp[KT-1] ----
        for nch in range(NCH):
            yp = ps.tile([128, C], F32, tag="st" if nch % 2 == 0 else "oa",
                         name=f"yp{nch}")
            nc.tensor.matmul(yp[:, 0:512],
                             onorm_sb[KT - 1][:, nch * 128:(nch + 1) * 128],
                             wp_sb[KT - 1][:, 0:512], start=True, stop=True)
            nc.tensor.matmul(yp[:, 512:768],
                             onorm_sb[KT - 1][:, nch * 128:(nch + 1) * 128],
                             wp_sb[KT - 1][:, 512:768], start=True, stop=True)
            ys = ysb_pool.tile([128, C], F32, tag="ysb", name=f"ys{nch}")
            nc.vector.tensor_add(ys[:], yp[:], ypart_sb[nch][:])
            eng = (nc.sync, nc.scalar, nc.gpsimd)[nch % 3]
            if nch < NCH - 1:
                eng.dma_start(y.ap()[nch * 128:(nch + 1) * 128, :], ys[:])
            else:
                nc.sync.dma_start(y.ap()[nch * 128:(nch + 1) * 128, 0:384],
                                  ys[:, 0:384])
                nc.scalar.dma_start(y.ap()[nch * 128:(nch + 1) * 128, 384:768],
                                    ys[:, 384:768])


_NC_CACHE = None


def _get_nc():
    global _NC_CACHE
    if _NC_CACHE is None:
        _NC_CACHE = _build()
    return _NC_CACHE


def make_in_maps(x, w_qkv, b_qkv, w_proj, b_proj):
    x = np.asarray(x, np.float32)
    w_qkv = np.ascontiguousarray(np.asarray(w_qkv, np.float32))
    b_qkv = np.asarray(b_qkv, np.float32)
    w_proj_b = np.ascontiguousarray(np.asarray(w_proj, np.float32))
    b_proj = np.asarray(b_proj, np.float32)

    w_qk = np.ascontiguousarray(w_qkv[:, :2 * C])
    w_v = np.ascontiguousarray(w_qkv[:, 2 * C:])
    b_qk = np.ascontiguousarray(b_qkv[:2 * C].reshape(2 * KT, 128, 1))
    b_vo = np.ascontiguousarray(
        np.stack([b_qkv[2 * C:], b_proj])).astype(np.float32)
    ones = np.ones((128, H), np.float32)

    in_maps = []
    for c in range(N_CORES):
        in_maps.append({
            "xT": np.ascontiguousarray(x[c].T),
            "w_qk": w_qk,
            "w_v": w_v,
            "w_proj": w_proj_b,
            "b_qk": b_qk,
            "b_vo": b_vo,
            "ones12": ones,
        })
    return in_maps


def kernel(x, w_qkv, b_qkv, w_proj, b_proj):
    nc = _get_nc()
    in_maps = make_in_maps(x, w_qkv, b_qkv, w_proj, b_proj)
    res = bass_utils.run_bass_kernel_spmd(nc, in_maps, list(range(N_CORES)))
    out = np.stack([res.results[c]["y"] for c in range(N_CORES)], axis=0)
    return out.astype(np.float32)


# revision 43
# speedup vs baseline: 1.0877x; 1.0877x over previous
"""Trainium2 Bass kernel for nn_Attention_84026740179215.

Multi-head attention: x[8,1024,768] -> qkv -> per-head softmax(QK^T/sqrt(d))V -> proj.
Sharding: pure data parallel, one batch element per NeuronCore (B=8 = 8 cores).

v3 layout (N=1024 tokens, C=768, H=12 heads, D=64):
  - Heads processed in PAIRS (2p, 2p+1). Q^T/K^T chunks [128, N] hold the even
    head in partitions 0:64 and the odd head in 64:128, so the two QK^T
    matmuls (K=64 each) run CONCURRENTLY on the PE via row tiling
    (tile_position (0,0) / (64,0)).
  - All matmul lhsT (stationary) operands are float32r -> self-loading
    matmuls, no separate LDWEIGHTS instruction on the PE sequencer.
    Moving operands that are never lhsT (q chunks, P=exp(S), w_v, w_proj)
    are bf16 to save SBUF/DMA.
  - exp via ACT (scale=1/8 fused) PSUM->SBUF per head tile [128, 1024].
  - V augmented with a ones column per head: PV (lhsT=vaug[m,65], rhs=P^T)
    also produces the softmax denominator Z in row 64.
  - Normalization by 1/Z via reciprocal_approx_fast + gpsimd
    partition_broadcast, written into the projection's lhsT layout [c', n].
  - QKV chunk production threads through the attention pairs as half-blocks
    ([128,512] PSUM accumulators borrowing the st slot rotation); V production
    runs just-in-time inside pair 0. The first two chunks (pair 0's Q,K) run
    kt-major off small per-kt weight DMAs so PE work starts ~2us in.
"""

import numpy as np
import ml_dtypes

import concourse.bacc as bacc
import concourse.bass as bass
import concourse.mybir as mybir
import concourse.tile as tile
from concourse import bass_utils

N_CORES = 8
N = 1024          # tokens per batch element
C = 768           # model dim
H = 12            # heads
D = 64            # head dim
KT = C // 128     # 6 k-tiles of the contraction dim
NCH = N // 128    # 8 chunks of the token dim (query side)
MT = N // 128     # 8 tiles of the token dim (key/value side)
NPAIR = H // 2    # 6 head pairs

DEBUG_TAPS = False
BENCH_ITERS = 0      # >0: wrap the body in a For_i loop (timing harness only)
F32 = mybir.dt.float32
F32R = mybir.dt.float32r
BF16 = mybir.dt.bfloat16
AF = mybir.ActivationFunctionType
NP_BF16 = ml_dtypes.bfloat16


def _build():
    nc = bacc.Bacc("TRN2", target_bir_lowering=False, debug=False,
                   num_devices=N_CORES)

    xT = nc.dram_tensor("xT", [C, N], F32R, kind="ExternalInput")
    w_qk = nc.dram_tensor("w_qk", [C, 2 * C], F32R, kind="ExternalInput")
    w_v = nc.dram_tensor("w_v", [C, C], F32R, kind="ExternalInput")
    w_proj = nc.dram_tensor("w_proj", [C, C], F32R, kind="ExternalInput")
    b_qk = nc.dram_tensor("b_qk", [2 * KT, 128, 1], F32, kind="ExternalInput")
    b_vo = nc.dram_tensor("b_vo", [2, C], F32, kind="ExternalInput")
    ones12 = nc.dram_tensor("ones12", [128, H], F32R, kind="ExternalInput")
    y = nc.dram_tensor("y", [N, C], F32, kind="ExternalOutput")
    dbg = {}

    with tile.TileContext(nc) as tc:
        if BENCH_ITERS > 0:
            with tc.For_i(0, BENCH_ITERS, 1):
                _body(nc, tc, xT, w_qk, w_v, w_proj, b_qk, b_vo, ones12, y, dbg)
        else:
            _body(nc, tc, xT, w_qk, w_v, w_proj, b_qk, b_vo, ones12, y, dbg)
    nc.compile()
    return nc


def _body(nc, tc, xT, w_qk, w_v, w_proj, b_qk, b_vo, ones12, y, dbg={}):
    import contextlib
    ctx = contextlib.ExitStack()
    with ctx:
        # ---- pools ----
        xt_pool = ctx.enter_context(tc.tile_pool(name="xt", bufs=1))
        wq_pool = ctx.enter_context(tc.tile_pool(name="wq", bufs=1))
        wv_pool = ctx.enter_context(tc.tile_pool(name="wv", bufs=1))
        qc_pool = ctx.enter_context(tc.tile_pool(name="qc", bufs=1))
        kc_pool = ctx.enter_context(tc.tile_pool(name="kc", bufs=1))
        pts_pool = ctx.enter_context(tc.tile_pool(name="pts", bufs=6))
        vaug_pool = ctx.enter_context(tc.tile_pool(name="vaug", bufs=1))
        onorm_pool = ctx.enter_context(tc.tile_pool(name="onorm", bufs=1))
        bias_pool = ctx.enter_context(tc.tile_pool(name="bias", bufs=1))
        u_pool = ctx.enter_context(tc.tile_pool(name="u", bufs=1))
        zrow_pool = ctx.enter_context(tc.tile_pool(name="zrow", bufs=1))
        rz_pool = ctx.enter_context(tc.tile_pool(name="rz", bufs=1))
        rzb_pool = ctx.enter_context(tc.tile_pool(name="rzb", bufs=1))
        ysb_pool = ctx.enter_context(tc.tile_pool(name="ysb", bufs=2))
        # PSUM: tag "st" 2x[128,1024]f32 + tag "oa" 2x[128,1024]f32 = all 8 banks
        ps = ctx.enter_context(tc.tile_pool(name="ps", bufs=2, space="PSUM"))

        # ---- input staging ----
        # Mini weight tiles for pair 0's chunks (cc=0 -> w_qk cols 0:128,
        # cc=KT -> cols 768:896): two plain [128, 128] DMAs per kt, first on
        # the queues so the kt-major bootstrap can start early.
        wmini = []
        xq = [nc.sync, nc.scalar, nc.gpsimd]
        for kt in range(KT):
            t = wq_pool.tile([128, 256], F32R, tag=f"wq{kt}", name=f"wm{kt}",
                             padded_shape=[128, 2 * C])
            xq[kt % 3].dma_start(t[:, 0:128],
                                 w_qk.ap()[kt * 128:(kt + 1) * 128, 0:128])
            xq[kt % 3].dma_start(t[:, 128:256],
                                 w_qk.ap()[kt * 128:(kt + 1) * 128, C:C + 128])
            wmini.append(t)
        # xT k-tiles, split in halves to pipeline the kt-major bootstrap.
        xt_sb = []
        for kt in range(KT):
            t = xt_pool.tile([128, N], F32R, tag=f"xt{kt}", name=f"xt{kt}")
            xq[kt % 3].dma_start(t[:, 0:512],
                                 xT.ap()[kt * 128:(kt + 1) * 128, 0:512])
            xq[(kt + 1) % 3].dma_start(t[:, 512:1024],
                                       xT.ap()[kt * 128:(kt + 1) * 128, 512:1024])
            xt_sb.append(t)
        # biases (b_qk per-partition scalars; b_v/b_o broadcast on-chip)
        bqk_sb = {}
        for cc in range(2 * KT):
            t = bias_pool.tile([128, 1], F32, tag=f"bqk{cc}", name=f"bqk{cc}")
            nc.gpsimd.dma_start(t[:], b_qk.ap()[cc])
            bqk_sb[cc] = t
        bv_row = bias_pool.tile([1, C], F32, tag="bvrow", name="bv_row")
        nc.gpsimd.dma_start(bv_row[:], b_vo.ap()[0:1, :])
        bo_row = bias_pool.tile([1, C], F32, tag="borow", name="bo_row")
        nc.gpsimd.dma_start(bo_row[:], b_vo.ap()[1:2, :])
        bv_sb = bias_pool.tile([128, C], F32, tag="bv", name="bv")
        nc.gpsimd.partition_broadcast(bv_sb[:], bv_row[:])
        bo_sb = bias_pool.tile([128, C], F32, tag="bo", name="bo")
        nc.gpsimd.partition_broadcast(bo_sb[:], bo_row[:])
        ones_sb = bias_pool.tile([128, H], F32R, tag="ones", name="ones")
        nc.gpsimd.dma_start(ones_sb[:], ones12.ap())
        # w_v k-tiles first (pair-0 V pass), then full w_qk tiles (their DMAs
        # block on the wmini slot until the bootstrap's kt stage reads finish).
        wv_sb = []
        for kt in range(KT):
            t2 = wv_pool.tile([128, C], F32R, tag=f"wv{kt}", name=f"wv{kt}")
            xq[(kt + 2) % 3].dma_start(
                t2[:], w_v.ap()[kt * 128:(kt + 1) * 128, :])
            wv_sb.append(t2)
        wq_sb = []
        for kt in range(KT):
            t = wq_pool.tile([128, 2 * C], F32R, tag=f"wq{kt}", name=f"wq{kt}")
            xq[(kt + 1) % 3].dma_start(
                t[:], w_qk.ap()[kt * 128:(kt + 1) * 128, :])
            wq_sb.append(t)

        # q and k chunks bf16: the row-tiled st matmuls need the LDWEIGHTS
        # path (tile_position), which silently breaks for f32r weights on HW.
        qc_sb = [qc_pool.tile([128, N], BF16, tag=f"qc{p}", name=f"qc{p}")
                 for p in range(NPAIR)]
        kc_sb = [kc_pool.tile([128, N], BF16, tag=f"kc{p}", name=f"kc{p}")
                 for p in range(NPAIR)]

        def chunk_half(cc, half, wsrc=None):
            """Produce chunk cc's columns [half*512:(half+1)*512]."""
            s = slice(half * 512, (half + 1) * 512)
            pc = ps.tile([128, 512], F32, tag="st", name=f"pc{cc}_{half}")
            for kt in range(KT):
                w = wsrc[kt] if wsrc else wq_sb[kt][:, cc * 128:(cc + 1) * 128]
                nc.tensor.matmul(pc[:], w, xt_sb[kt][:, s],
                                 start=(kt == 0), stop=(kt == KT - 1))
            dst = qc_sb[cc] if cc < KT else kc_sb[cc - KT]
            nc.vector.tensor_scalar_add(dst[:, s], pc[:], bqk_sb[cc][:])

        vaug_sb = [None] * MT

        def make_vaug(mt):
            """V rows for m-tile mt, augmented with a ones column per head."""
            vc = ps.tile([128, C], F32, tag="st", name=f"vc{mt}")
            for kt in range(KT):
                nc.tensor.matmul(vc[:, 0:512],
                                 xt_sb[kt][:, mt * 128:(mt + 1) * 128],
                                 wv_sb[kt][:, 0:512],
                                 start=(kt == 0), stop=(kt == KT - 1))
                nc.tensor.matmul(vc[:, 512:768],
                                 xt_sb[kt][:, mt * 128:(mt + 1) * 128],
                                 wv_sb[kt][:, 512:768],
                                 start=(kt == 0), stop=(kt == KT - 1))
            va = vaug_pool.tile([128, H * (D + 1)], F32R, tag=f"vaug{mt}",
                                name=f"vaug{mt}")
            va_h = va[:].rearrange("p (h s) -> p h s", h=H)
            nc.vector.tensor_copy(va_h[:, :, D], ones_sb[:])
            nc.vector.tensor_add(
                va_h[:, :, 0:D],
                vc[:].rearrange("p (h s) -> p h s", h=H),
                bv_sb[:].rearrange("p (h s) -> p h s", h=H))
            vaug_sb[mt] = va

        onorm_sb = [onorm_pool.tile([128, N], F32R, tag=f"onorm{p}",
                                    name=f"onorm{p}")
                    for p in range(NPAIR)]

        class Pair:
            """Heads h0=2p (partitions 0:64) and h1=2p+1 (64:128)."""

            def __init__(self, p):
                self.p = p
                self.qt = qc_sb[p]
                self.kt = kc_sb[p]
                self.oa = [ps.tile([128, N], F32, tag="oa", name=f"oa{2*p+i}")
                           for i in range(2)]
                self.pts = [[None] * MT, [None] * MT]

            def st_exp(self, mc):
                st = [ps.tile([128, N], F32, tag="st",
                              name=f"st{2*self.p+i}_{mc}") for i in range(2)]
                ms = slice(mc * 128, (mc + 1) * 128)
                for half in range(2):
                    s = slice(half * 512, (half + 1) * 512)
                    nc.tensor.matmul(st[0][:, s], self.kt[0:64, ms],
                                     self.qt[0:64, s], start=True, stop=True,
                                     tile_position=(0, 0))
                    nc.tensor.matmul(st[1][:, s], self.kt[64:128, ms],
                                     self.qt[64:128, s], start=True, stop=True,
                                     tile_position=(64, 0))
                for i in range(2):
                    pt = pts_pool.tile([128, N], F32R, tag="pts",
                                       name=f"pt{2*self.p+i}_{mc}")
                    nc.scalar.activation(pt[:], st[i][:], AF.Exp,
                                         scale=float(D) ** -0.5)
                    self.pts[i][mc] = pt

            def pv(self, mc):
                for i in range(2):
                    h = 2 * self.p + i
                    for half in range(2):
                        s = slice(half * 512, (half + 1) * 512)
                        nc.tensor.matmul(
                            self.oa[i][0:D + 1, s],
                            vaug_sb[mc][:, h * (D + 1):(h + 1) * (D + 1)],
                            self.pts[i][mc][:, s],
                            start=(mc == 0), stop=(mc == MT - 1))

            def norm(self, i):
                h, oa = 2 * self.p + i, self.oa[i]
                u = u_pool.tile([D, N], BF16, tag="u", name=f"u{h}")
                nc.vector.tensor_copy(u[:], oa[0:D, :])
                zrow = zrow_pool.tile([1, N], F32, tag="zrow", name=f"z{h}")
                nc.vector.tensor_copy(zrow[:], oa[D:D + 1, :])
                rz = rz_pool.tile([1, N], F32, tag="rz", name=f"rz{h}")
                nc.vector.reciprocal_approx_fast(rz[:], zrow[:])
                rzb = rzb_pool.tile([D, N], F32, tag="rzb", name=f"rzb{h}")
                nc.gpsimd.partition_broadcast(rzb[:], rz[:])
                nc.vector.tensor_mul(
                    onorm_sb[self.p][i * D:(i + 1) * D, :], u[:], rzb[:])

        def do_pair(p, fillers=(), lookahead=2):
            # fillers: per-mc GROUPS of callables, emitted back-to-back so the
            # 2-slot "st" rotation keeps its parity (a lone filler would make
            # the next st matmul wait on the later of the two in-flight ACTs)
            pr = Pair(p)
            fi = iter(fillers)
            for mc in range(MT):
                pr.st_exp(mc)
                for f in next(fi, ()):
                    f()
                if mc >= lookahead:
                    pr.pv(mc - lookahead)
            for grp in fi:
                for f in grp:
                    f()
            for mc in range(MT - lookahead, MT):
                pr.pv(mc)
            pr.norm(0)
            pr.norm(1)
            return pr

        # ---- bootstrap: pair 0's chunks kt-major off the mini weight DMAs ----
        # three accumulators in flight: (0,h0)+(0,h1) on "st", (KT,h0) on "oa"
        bpc = {}
        bpc[(0, 0)] = ps.tile([128, 512], F32, tag="st", name="bpc00")
        bpc[(0, 1)] = ps.tile([128, 512], F32, tag="st", name="bpc01")
        bpc[(KT, 0)] = ps.tile([128, 512], F32, tag="oa", name="bpc60")
        for kt in range(KT):
            for (cc, half), pc in bpc.items():
                s = slice(half * 512, (half + 1) * 512)
                w = wmini[kt][:, 0:128] if cc == 0 else wmini[kt][:, 128:256]
                nc.tensor.matmul(pc[:], w, xt_sb[kt][:, s],
                                 start=(kt == 0), stop=(kt == KT - 1))
        for (cc, half), pc in bpc.items():
            dst = qc_sb[cc] if cc < KT else kc_sb[cc - KT]
            s = slice(half * 512, (half + 1) * 512)
            nc.vector.tensor_scalar_add(dst[:, s], pc[:], bqk_sb[cc][:])

        # pair 0: V production JIT + remaining chunk halves as PE fillers,
        # grouped in parity-preserving pairs
        V = lambda mt: (lambda: make_vaug(mt))
        CH = lambda cc, h: (lambda: chunk_half(cc, h))
        f0 = [(), (CH(KT, 1), V(0)), (V(1), V(2)), (V(3), V(4)),
              (V(5), V(6)), (V(7), CH(1, 0)), (CH(1, 1), CH(KT + 1, 0))]
        do_pair(0, f0)

        # w_proj staging: reuses the wq slots (frees after the last chunk
        # reads wq{kt}, i.e. during pair 4; proj needs them in pair 5+).
        wp_sb = []
        for kt in range(KT):
            t = wq_pool.tile([128, C], F32R, tag=f"wq{kt}", name=f"wp{kt}",
                             padded_shape=[128, 2 * C])
            nc.sync.dma_start(t[:], w_proj.ap()[kt * 128:(kt + 1) * 128, :])
            wp_sb.append(t)

        # steady pairs: pair p fillers produce K(p)h1 + Q(p+1) + K(p+1)h0
        for p in range(1, NPAIR):
            if p + 1 < NPAIR:
                fillers = [(), (CH(KT + p, 1), CH(p + 1, 0)),
                           (), (CH(p + 1, 1), CH(KT + p + 1, 0))]
            else:
                fillers = [(), (CH(KT + p, 1),)]
            do_pair(p, fillers, lookahead=2 if p < NPAIR - 1 else 3)

        # ---- projection y[n, c] ----
        for nch in range(NCH):
            yp = ps.tile([128, C], F32, tag="st" if nch % 2 == 0 else "oa",
                         name=f"yp{nch}")
            for kt in range(KT):
                nc.tensor.matmul(yp[:, 0:512],
                                 onorm_sb[kt][:, nch * 128:(nch + 1) * 128],
                                 wp_sb[kt][:, 0:512],
                                 start=(kt == 0), stop=(kt == KT - 1))
                nc.tensor.matmul(yp[:, 512:768],
                                 onorm_sb[kt][:, nch * 128:(nch + 1) * 128],
                                 wp_sb[kt][:, 512:768],
                                 start=(kt == 0), stop=(kt == KT - 1))
            ys = ysb_pool.tile([128, C], F32, tag="ysb", name=f"ys{nch}")
            nc.vector.tensor_add(ys[:], yp[:], bo_sb[:])
            eng = (nc.sync, nc.scalar, nc.gpsimd)[nch % 3]
            if nch < NCH - 1:
                eng.dma_start(y.ap()[nch * 128:(nch + 1) * 128, :], ys[:])
            else:
                nc.sync.dma_start(y.ap()[nch * 128:(nch + 1) * 128, 0:384],
                                  ys[:, 0:384])
                nc.scalar.dma_start(y.ap()[nch * 128:(nch + 1) * 128, 384:768],
                                    ys[:, 384:768])


_NC_CACHE = None


def _get_nc():
    global _NC_CACHE
    if _NC_CACHE is None:
        _NC_CACHE = _build()
    return _NC_CACHE


def make_in_maps(x, w_qkv, b_qkv, w_proj, b_proj):
    x = np.asarray(x, np.float32)
    w_qkv = np.ascontiguousarray(np.asarray(w_qkv, np.float32))
    b_qkv = np.asarray(b_qkv, np.float32)
    w_proj_b = np.ascontiguousarray(np.asarray(w_proj, np.float32))
    b_proj = np.asarray(b_proj, np.float32)

    w_qk = np.ascontiguousarray(w_qkv[:, :2 * C])
    w_v = np.ascontiguousarray(w_qkv[:, 2 * C:])
    b_qk = np.ascontiguousarray(b_qkv[:2 * C].reshape(2 * KT, 128, 1))
    b_vo = np.ascontiguousarray(
        np.stack([b_qkv[2 * C:], b_proj])).astype(np.float32)
    ones = np.ones((128, H), np.float32)

    in_maps = []
    for c in range(N_CORES):
        in_maps.append({
            "xT": np.ascontiguousarray(x[c].T),
            "w_qk": w_qk,
            "w_v": w_v,
            "w_proj": w_proj_b,
            "b_qk": b_qk,
            "b_vo": b_vo,
            "ones12": ones,
        })
    return in_maps


def kernel(x, w_qkv, b_qkv, w_proj, b_proj):
    nc = _get_nc()
    in_maps = make_in_maps(x, w_qkv, b_qkv, w_proj, b_proj)
    res = bass_utils.run_bass_kernel_spmd(nc, in_maps, list(range(N_CORES)))
    out = np.stack([res.results[c]["y"] for c in range(N_CORES)], axis=0)
    return out.astype(np.float32)
